# revision 31
# baseline (speedup 1.0000x reference)
import sys

if "/opt/trn_rl_repo" not in sys.path:
    sys.path.insert(0, "/opt/trn_rl_repo")

import ml_dtypes
import numpy as np

import concourse.bacc as bacc
import concourse.bass as bass
import concourse.mybir as mybir
import concourse.tile as tile
from concourse.bass_utils import run_bass_kernel_spmd

# Problem constants (hardcoded per contract)
B, S, H = 4, 4096, 2048
HH = H // 2  # 1024
HS = HH // 8  # 128 hidden columns handled per core
RANKS = [4, 8, 16]
SCALING = 16.0 / max(RANKS)  # 1.0
RESIDUAL_SCALE = 1.0
THR = [0.3, 0.7]
N_CORES = 8
R = (B * S) // N_CORES  # 2048 rows per core
P = 128
NT = R // P  # 16 row tiles per core
KC = H // P  # 16 col chunks
RP = 32  # padded concat rank (4+8+16=28 -> 32)
NRES = 10  # keys row-tiles kept SBUF-resident for the phase-C residual add
NVPRE = 3  # values tiles whose transpose+x@A is emitted before the mask
F32 = mybir.dt.float32
F32R = mybir.dt.float32r
BF16 = mybir.dt.bfloat16
BF16NP = np.dtype(ml_dtypes.bfloat16)

_cache = {}


def _build_program():
    nc = bacc.Bacc("TRN2", target_bir_lowering=False, debug=False,
                   num_devices=N_CORES)

    k_slab = nc.dram_tensor("k_slab", [R, H], F32R, kind="ExternalInput").ap()
    v_slab = nc.dram_tensor("v_slab", [R, H], F32R, kind="ExternalInput").ap()
    w1v = nc.dram_tensor("w1v", [P, KC * HS], F32, kind="ExternalInput").ap()
    b1s = nc.dram_tensor("b1s", [HS, 1], F32, kind="ExternalInput").ap()
    w2s = nc.dram_tensor("w2s", [HS, 1], F32, kind="ExternalInput").ap()
    b2 = nc.dram_tensor("b2", [1, 1], F32, kind="ExternalInput").ap()
    akbd = nc.dram_tensor("akbd", [P, KC * RP], BF16,
                          kind="ExternalInput").ap()
    avbd = nc.dram_tensor("avbd", [P, KC * RP], BF16,
                          kind="ExternalInput").ap()
    bkbd = nc.dram_tensor("bkbd", [RP, H], BF16, kind="ExternalInput").ap()
    bvbd = nc.dram_tensor("bvbd", [RP, H], BF16, kind="ExternalInput").ap()
    fsel = nc.dram_tensor("fsel", [N_CORES, B], F32, kind="ExternalInput").ap()
    maskc = nc.dram_tensor("maskc", [1, 3 * RP], F32,
                           kind="ExternalInput").ap()
    idm = nc.dram_tensor("idm", [P, P], F32R, kind="ExternalInput").ap()
    onesd = nc.dram_tensor("onesd", [P, 1], F32R, kind="ExternalInput").ap()
    ck_slab = nc.dram_tensor("ck_slab", [R, H], F32, kind="ExternalOutput").ap()
    cv_slab = nc.dram_tensor("cv_slab", [R, H], F32, kind="ExternalOutput").ap()

    def emit_txa(xt, xtt, trp, tp, a_sb, t_all, tslot, id128):
        """Transpose xt and accumulate (x@A)^T into t_all[:, slot]."""
        for g in range(4):
            tr = trp.tile([P, 512], F32R, tag="tr")
            for j in range(4):
                k = g * 4 + j
                nc.tensor.transpose(tr[:, j * P:(j + 1) * P],
                                    xt[:, k * P:(k + 1) * P],
                                    id128[:])
            if g < 3:
                nc.scalar.copy(xtt[:, g * 512:(g + 1) * 512],
                               tr[:].bitcast(F32))
            else:
                nc.vector.tensor_copy(xtt[:, g * 512:(g + 1) * 512],
                                      tr[:].bitcast(F32))
        ps_t = tp.tile([RP, P], F32, tag="t")
        for k in range(KC):
            nc.tensor.matmul(ps_t[:], a_sb[:, k * RP:(k + 1) * RP],
                             xtt[:, k * P:(k + 1) * P],
                             start=(k == 0), stop=(k == KC - 1))
        nc.vector.tensor_copy(t_all[:, tslot * P:(tslot + 1) * P], ps_t[:])

    def emit_out(xt, t_all, tslot, bm, pso, oph, o_dram, t):
        """out tile = xt + t@Bmask, staged in half-tiles, DMA'd out."""
        for half in range(2):
            oh = oph.tile([P, 1024], F32, tag="oh")
            for n2 in range(2):
                n = half * 2 + n2
                ps_o = pso.tile([P, 512], F32, tag="o")
                nc.tensor.matmul(ps_o[:],
                                 t_all[:, tslot * P:(tslot + 1) * P],
                                 bm[:, n * 512:(n + 1) * 512],
                                 start=True, stop=True)
                eng = nc.vector
                eng.tensor_tensor(oh[:, n2 * 512:(n2 + 1) * 512],
                                  ps_o[:],
                                  xt[:, n * 512:(n + 1) * 512].bitcast(F32),
                                  op=mybir.AluOpType.add)
            nc.scalar.dma_start(
                out=o_dram[t * P:(t + 1) * P,
                           half * 1024:(half + 1) * 1024],
                in_=oh[:])

    with tile.TileContext(nc) as tc:
        with tc.tile_pool(name="const", bufs=1) as const:
            ones128 = const.tile([P, 1], F32R)
            nc.sync.dma_start(out=ones128[:], in_=onesd[:])
            id128 = const.tile([P, P], F32R)
            nc.sync.dma_start(out=id128[:], in_=idm[:])
            quarter = const.tile([B, 1], F32)
            nc.vector.memset(quarter[:], 1.0 / B)
            one1 = const.tile([1, 1], F32)
            nc.vector.memset(one1[:], 1.0)
            fsel_sb = const.tile([N_CORES, B], F32)
            nc.gpsimd.dma_start(out=fsel_sb[:], in_=fsel[:])
            maskc_sb = const.tile([1, 3 * RP], F32)
            nc.gpsimd.dma_start(out=maskc_sb[:], in_=maskc[:])
            b2_sb = const.tile([B, 1], F32)
            for p in range(B):
                nc.gpsimd.dma_start(out=b2_sb[p:p + 1, :], in_=b2[:])
            # per-core MLP slice params
            w1s_sb = const.tile([P, KC * HS], F32)
            nc.sync.dma_start(out=w1s_sb[:], in_=w1v[:])
            b1s_sb = const.tile([HS, 1], F32)
            nc.gpsimd.dma_start(out=b1s_sb[:], in_=b1s[:])
            w2s_sb = const.tile([HS, 1], F32)
            nc.gpsimd.dma_start(out=w2s_sb[:], in_=w2s[:])
            # LoRA A and B matrices, concatenated, bf16 (host-prepared)
            akb = const.tile([P, KC * RP], BF16)
            nc.sync.dma_start(out=akb[:], in_=akbd[:])
            avb = const.tile([P, KC * RP], BF16)
            nc.sync.dma_start(out=avb[:], in_=avbd[:])
            bkb = const.tile([RP, H], BF16)
            bvb = const.tile([RP, H], BF16)
            nc.gpsimd.dma_start(out=bkb[:], in_=bkbd[:])
            nc.gpsimd.dma_start(out=bvb[:], in_=bvbd[:])
            bmk = const.tile([RP, H], BF16)
            bmv = const.tile([RP, H], BF16)
            # (x@A)^T per tile, bf16
            tk_all = const.tile([RP, NT * P], BF16)
            tv_all = const.tile([RP, NT * P], BF16)
            kres = [const.tile([P, H], F32R, tag=f"kr{t}", name=f"kr{t}")
                    for t in range(NRES)]
            partial_sb = const.tile([1, H], F32)
            gath_sb = const.tile([N_CORES, H], F32)
            xmt_sb = const.tile([P, KC * B], F32)
            hb_sb = const.tile([HS, B], F32)
            psum_imp_sb = const.tile([B, 1], F32)
            imp_sb = const.tile([B, 1], F32)
            avg_sb = const.tile([1, 1], F32)
            s1_sb = const.tile([1, 1], F32)
            s2_sb = const.tile([1, 1], F32)
            m1_sb = const.tile([1, RP], F32)
            m2_sb = const.tile([1, RP], F32)
            mask_sb = const.tile([1, RP], F32)
            maskt_sb = const.tile([RP, 1], F32)

            vsp_cm = tc.tile_pool(name="vsp", bufs=4)
            vsp = vsp_cm.__enter__()
            oph_cm = tc.tile_pool(name="oph", bufs=4)
            oph = oph_cm.__enter__()
            vpre = []  # values tiles pre-loaded before the mask is ready

            # ---- Phase A1: stream keys, colsum all tiles, and
            # transpose+x@A for the non-resident (streamed) tiles ----
            with tc.tile_pool(name="ksp", bufs=2) as ksp, \
                 tc.tile_pool(name="xttp", bufs=2) as xttp, \
                 tc.tile_pool(name="csp", bufs=4, space="PSUM") as csp, \
                 tc.tile_pool(name="trp", bufs=2, space="PSUM") as trp, \
                 tc.tile_pool(name="tp", bufs=2, space="PSUM") as tp:
                cs = [csp.tile([1, 512], F32, tag="cs", name=f"cs{n}")
                      for n in range(4)]
                # streamed tiles interleaved with resident colsums so
                # the last colsum lands right after the last keys DMA
                nstream = NT - NRES
                order = []
                kq = list(range(NRES))
                for si, t in enumerate(range(NRES, NT)):
                    order.append(t)
                    take = (NRES * (si + 1)) // nstream - (NRES * si) // nstream
                    for _ in range(take):
                        order.append(kq.pop(0))
                order += kq
                first = {"v": True}
                ncs = {"n": 0}
                for t in order:
                    if t < NRES:
                        xt = kres[t]
                    else:
                        xt = ksp.tile([P, H], F32R, tag="ks")
                    nc.sync.dma_start(out=xt[:],
                                      in_=k_slab[t * P:(t + 1) * P, :])
                    ncs["n"] += 1
                    for n in range(4):
                        nc.tensor.matmul(
                            cs[n][:], ones128[:],
                            xt[:, n * 512:(n + 1) * 512],
                            start=first["v"], stop=(ncs["n"] == NT),
                            skip_group_check=True)
                    first["v"] = False
                    if t >= NRES:
                        xtt = xttp.tile([P, H], BF16, tag="xtt")
                        emit_txa(xt, xtt, trp, tp, akb, tk_all, t, id128)
                for n in range(4):
                    nc.scalar.copy(partial_sb[:, n * 512:(n + 1) * 512],
                                   cs[n][:])

                # ---- Collective 1: AllGather partial colsums ----
                with tc.tile_pool(name="dram", bufs=1, space="DRAM") as dram:
                    cc_in = dram.tile([1, H], F32)
                    cc_out = dram.tile([N_CORES, H], F32)
                    nc.gpsimd.dma_start(out=cc_in[:], in_=partial_sb[:])
                    nc.gpsimd.collective_compute(
                        "AllGather", mybir.AluOpType.bypass,
                        replica_groups=[list(range(N_CORES))],
                        ins=[cc_in.opt()], outs=[cc_out.opt()])
                    nc.gpsimd.dma_start(out=gath_sb[:], in_=cc_out[:])

                # ---- Phase A2 (overlaps collective): transpose+x@A for
                # the resident keys tiles ----
                for t in range(NRES):
                    xtt = xttp.tile([P, H], BF16, tag="xtt")
                    emit_txa(kres[t], xtt, trp, tp, akb, tk_all, t, id128)

                # ---- values pre-work (mask-independent) ----
                for t in range(NVPRE):
                    xt = vsp.tile([P, H], F32R, tag="vs")
                    nc.sync.dma_start(out=xt[:],
                                      in_=v_slab[t * P:(t + 1) * P, :])
                    vpre.append(xt)
                    xtt = xttp.tile([P, H], BF16, tag="xtt")
                    emit_txa(xt, xtt, trp, tp, avb, tv_all, t, id128)

                # ---- Phase B: MLP slice + collective 2 + mask ----
                with tc.tile_pool(name="mlp", bufs=2, space="PSUM") as mlp:
                    for k in range(KC):
                        ps = mlp.tile([P, B], F32, tag="xm")
                        nc.tensor.matmul(ps[:],
                                         gath_sb[:, k * P:(k + 1) * P],
                                         fsel_sb[:],
                                         start=True, stop=True)
                        nc.vector.tensor_copy(xmt_sb[:, k * B:(k + 1) * B],
                                              ps[:])
                    ps_h = mlp.tile([HS, B], F32, tag="h")
                    for k in range(KC):
                        nc.tensor.matmul(
                            ps_h[:], w1s_sb[:, k * HS:(k + 1) * HS],
                            xmt_sb[:, k * B:(k + 1) * B],
                            start=(k == 0), stop=(k == KC - 1))
                    nc.scalar.activation(hb_sb[:], ps_h[:],
                                         mybir.ActivationFunctionType.Relu,
                                         bias=b1s_sb[:, 0:1])
                    ps_p = mlp.tile([B, 1], F32, tag="p")
                    nc.tensor.matmul(ps_p[:], hb_sb[:], w2s_sb[:],
                                     start=True, stop=True)
                    nc.scalar.copy(psum_imp_sb[:], ps_p[:])

                    with tc.tile_pool(name="dram2", bufs=1,
                                      space="DRAM") as dram2:
                        cc2_in = dram2.tile([B, 1], F32)
                        cc2_out = dram2.tile([B, 1], F32)
                        nc.gpsimd.dma_start(out=cc2_in[:],
                                            in_=psum_imp_sb[:])
                        nc.gpsimd.collective_compute(
                            "AllReduce", mybir.AluOpType.add,
                            replica_groups=[list(range(N_CORES))],
                            ins=[cc2_in.opt()], outs=[cc2_out.opt()])
                        nc.gpsimd.dma_start(out=imp_sb[:], in_=cc2_out[:])

                    nc.vector.tensor_scalar(imp_sb[:], imp_sb[:],
                                            b2_sb[:, 0:1], None,
                                            op0=mybir.AluOpType.add)
                    nc.scalar.activation(imp_sb[:], imp_sb[:],
                                         mybir.ActivationFunctionType.Sigmoid)
                    ps_a = mlp.tile([1, 1], F32, tag="avg")
                    nc.tensor.matmul(ps_a[:], imp_sb[:], quarter[:],
                                     start=True, stop=True)
                    nc.scalar.copy(avg_sb[:], ps_a[:])
                    nc.vector.tensor_scalar(s1_sb[:], avg_sb[:], THR[0],
                                            None, op0=mybir.AluOpType.is_ge)
                    nc.vector.tensor_scalar(s2_sb[:], avg_sb[:], THR[1],
                                            None, op0=mybir.AluOpType.is_ge)
                    nc.vector.tensor_scalar(m1_sb[:], maskc_sb[:, RP:2 * RP],
                                            s1_sb[0:1, 0:1], None,
                                            op0=mybir.AluOpType.mult)
                    nc.vector.tensor_scalar(m2_sb[:],
                                            maskc_sb[:, 2 * RP:3 * RP],
                                            s2_sb[0:1, 0:1], None,
                                            op0=mybir.AluOpType.mult)
                    nc.vector.tensor_tensor(mask_sb[:], maskc_sb[:, 0:RP],
                                            m1_sb[:], op=mybir.AluOpType.add)
                    nc.vector.tensor_tensor(mask_sb[:], mask_sb[:], m2_sb[:],
                                            op=mybir.AluOpType.add)
                    ps_mt = mlp.tile([RP, 1], F32, tag="mt")
                    nc.tensor.matmul(ps_mt[:], mask_sb[:], one1[:],
                                     start=True, stop=True)
                    nc.scalar.copy(maskt_sb[:], ps_mt[:])
                    nc.vector.tensor_scalar(bmk[:], bkb[:],
                                            maskt_sb[:, 0:1], None,
                                            op0=mybir.AluOpType.mult)
                    nc.vector.tensor_scalar(bmv[:], bvb[:],
                                            maskt_sb[:, 0:1], None,
                                            op0=mybir.AluOpType.mult)

            # ---- Phase C: keys outputs ----
            with tc.tile_pool(name="krr", bufs=2) as krr, \
                 tc.tile_pool(name="kpso", bufs=2, space="PSUM") as kpso:
                for t in range(NT):
                    if t < NRES:
                        xt = kres[t]
                    else:
                        xt = krr.tile([P, H], F32R, tag="krr")
                        nc.gpsimd.dma_start(out=xt[:],
                                            in_=k_slab[t * P:(t + 1) * P, :])
                    emit_out(xt, tk_all, t, bmk, kpso, oph, ck_slab, t)

            # ---- Phase D: values outputs (and remaining pre-work) ----
            with tc.tile_pool(name="vxttp", bufs=2) as vxttp, \
                 tc.tile_pool(name="vtrp", bufs=2, space="PSUM") as vtrp, \
                 tc.tile_pool(name="vtp", bufs=2, space="PSUM") as vtp, \
                 tc.tile_pool(name="vpso", bufs=2, space="PSUM") as vpso:
                for t in range(NVPRE):
                    emit_out(vpre[t], tv_all, t, bmv, vpso, oph, cv_slab, t)
                for t in range(NVPRE, NT):
                    xt = vsp.tile([P, H], F32R, tag="vs")
                    nc.sync.dma_start(out=xt[:],
                                      in_=v_slab[t * P:(t + 1) * P, :])
                    xtt = vxttp.tile([P, H], BF16, tag="vxtt")
                    emit_txa(xt, xtt, vtrp, vtp, avb, tv_all, t, id128)
                    emit_out(xt, tv_all, t, bmv, vpso, oph, cv_slab, t)
            oph_cm.__exit__(None, None, None)
            vsp_cm.__exit__(None, None, None)

    nc.compile()
    return nc


def _get_program():
    if "nc" not in _cache:
        _cache["nc"] = _build_program()
    return _cache["nc"]


def _prep_in_maps(inputs):
    f32 = np.float32
    keys = np.asarray(inputs["keys"], dtype=f32)
    values = np.asarray(inputs["values"], dtype=f32)
    kf = np.ascontiguousarray(keys.reshape(B * S, H))
    vf = np.ascontiguousarray(values.reshape(B * S, H))
    scale = SCALING * RESIDUAL_SCALE

    def cat_a(a0, a1, a2):
        out = np.zeros((H, RP), dtype=f32)
        out[:, 0:4] = a0
        out[:, 4:12] = a1
        out[:, 12:28] = a2
        # chunk layout: [128, KC*RP], row p col k*RP+r = A[k*128+p, r]
        return np.ascontiguousarray(
            out.reshape(KC, P, RP).transpose(1, 0, 2).reshape(P, KC * RP)
        ).astype(BF16NP)

    def cat_b(b0, b1_, b2_):
        out = np.zeros((RP, H), dtype=f32)
        out[0:4, :] = b0
        out[4:12, :] = b1_
        out[12:28, :] = b2_
        return np.ascontiguousarray(out * scale).astype(BF16NP)

    acat_k = cat_a(inputs["kA0"], inputs["kA1"], inputs["kA2"])
    acat_v = cat_a(inputs["vA0"], inputs["vA1"], inputs["vA2"])
    bcat_k = cat_b(inputs["kB0"], inputs["kB1"], inputs["kB2"])
    bcat_v = cat_b(inputs["vB0"], inputs["vB1"], inputs["vB2"])

    fsel = np.zeros((N_CORES, B), dtype=f32)
    for c in range(N_CORES):
        fsel[c, c // 2] = 1.0 / S

    u = np.zeros((3, RP), dtype=f32)
    u[0, 0:4] = 1.0
    u[1, 4:12] = 1.0
    u[2, 12:28] = 1.0
    maskc = np.concatenate([u[0], u[1] - u[0], u[2] - u[1]]).astype(f32)

    w1 = np.ascontiguousarray(inputs["w1"], dtype=f32)
    b1 = np.asarray(inputs["b1"], dtype=f32).reshape(HH)
    w2 = np.asarray(inputs["w2"], dtype=f32).reshape(HH)

    common = {
        "b2": np.ascontiguousarray(
            np.asarray(inputs["b2"], dtype=f32).reshape(1, 1)),
        "akbd": acat_k, "avbd": acat_v,
        "bkbd": bcat_k, "bvbd": bcat_v,
        "fsel": fsel, "maskc": maskc.reshape(1, 3 * RP),
        "idm": np.eye(P, dtype=f32), "onesd": np.ones((P, 1), dtype=f32),
    }
    out = []
    for c in range(N_CORES):
        w1c = w1[:, c * HS:(c + 1) * HS]  # [H, HS]
        w1vc = np.ascontiguousarray(
            w1c.reshape(KC, P, HS).transpose(1, 0, 2).reshape(P, KC * HS))
        out.append(dict(
            common,
            k_slab=np.ascontiguousarray(kf[c * R:(c + 1) * R]),
            v_slab=np.ascontiguousarray(vf[c * R:(c + 1) * R]),
            w1v=w1vc,
            b1s=np.ascontiguousarray(b1[c * HS:(c + 1) * HS].reshape(HS, 1)),
            w2s=np.ascontiguousarray(w2[c * HS:(c + 1) * HS].reshape(HS, 1)),
        ))
    return out


def kernel(**inputs):
    in_maps = _prep_in_maps(inputs)
    nc = _get_program()
    res = run_bass_kernel_spmd(nc, in_maps, list(range(N_CORES)),
                               **_cache.get("run_kwargs", {}))
    _cache["last_result"] = res
    ck = np.concatenate([res.results[c]["ck_slab"] for c in range(N_CORES)],
                        axis=0).reshape(B, S, H)
    cv = np.concatenate([res.results[c]["cv_slab"] for c in range(N_CORES)],
                        axis=0).reshape(B, S, H)
    return ck, cv


# revision 32
# speedup vs baseline: 1.0056x; 1.0056x over previous
import sys

if "/opt/trn_rl_repo" not in sys.path:
    sys.path.insert(0, "/opt/trn_rl_repo")

import ml_dtypes
import numpy as np

import concourse.bacc as bacc
import concourse.bass as bass
import concourse.mybir as mybir
import concourse.tile as tile
from concourse.bass_utils import run_bass_kernel_spmd

# Problem constants (hardcoded per contract)
B, S, H = 4, 4096, 2048
HH = H // 2  # 1024
HS = HH // 8  # 128 hidden columns handled per core
RANKS = [4, 8, 16]
SCALING = 16.0 / max(RANKS)  # 1.0
RESIDUAL_SCALE = 1.0
THR = [0.3, 0.7]
N_CORES = 8
R = (B * S) // N_CORES  # 2048 rows per core
P = 128
NT = R // P  # 16 row tiles per core
KC = H // P  # 16 col chunks
RP = 32  # padded concat rank (4+8+16=28 -> 32)
NRES = 10  # keys row-tiles kept SBUF-resident for the phase-C residual add
NVPRE = 3  # values tiles whose transpose+x@A is emitted before the mask
F32 = mybir.dt.float32
F32R = mybir.dt.float32r
BF16 = mybir.dt.bfloat16
BF16NP = np.dtype(ml_dtypes.bfloat16)

_cache = {}


def _build_program():
    nc = bacc.Bacc("TRN2", target_bir_lowering=False, debug=False,
                   num_devices=N_CORES)

    k_slab = nc.dram_tensor("k_slab", [R, H], F32R, kind="ExternalInput").ap()
    v_slab = nc.dram_tensor("v_slab", [R, H], F32R, kind="ExternalInput").ap()
    w1v = nc.dram_tensor("w1v", [P, KC * HS], F32, kind="ExternalInput").ap()
    b1s = nc.dram_tensor("b1s", [HS, 1], F32, kind="ExternalInput").ap()
    w2s = nc.dram_tensor("w2s", [HS, 1], F32, kind="ExternalInput").ap()
    b2 = nc.dram_tensor("b2", [1, 1], F32, kind="ExternalInput").ap()
    akbd = nc.dram_tensor("akbd", [P, KC * RP], BF16,
                          kind="ExternalInput").ap()
    avbd = nc.dram_tensor("avbd", [P, KC * RP], BF16,
                          kind="ExternalInput").ap()
    bkbd = nc.dram_tensor("bkbd", [RP, H], BF16, kind="ExternalInput").ap()
    bvbd = nc.dram_tensor("bvbd", [RP, H], BF16, kind="ExternalInput").ap()
    fsel = nc.dram_tensor("fsel", [N_CORES, B], F32, kind="ExternalInput").ap()
    maskc = nc.dram_tensor("maskc", [1, 3 * RP], F32,
                           kind="ExternalInput").ap()
    idm = nc.dram_tensor("idm", [P, P], F32R, kind="ExternalInput").ap()
    onesd = nc.dram_tensor("onesd", [P, 1], F32R, kind="ExternalInput").ap()
    ck_slab = nc.dram_tensor("ck_slab", [R, H], F32, kind="ExternalOutput").ap()
    cv_slab = nc.dram_tensor("cv_slab", [R, H], F32, kind="ExternalOutput").ap()

    def emit_txa(xt, xtt, trp, tp, a_sb, t_all, tslot, id128):
        """Transpose xt and accumulate (x@A)^T into t_all[:, slot]."""
        for g in range(4):
            tr = trp.tile([P, 512], F32R, tag="tr")
            for j in range(4):
                k = g * 4 + j
                nc.tensor.transpose(tr[:, j * P:(j + 1) * P],
                                    xt[:, k * P:(k + 1) * P],
                                    id128[:])
            if g < 3:
                nc.scalar.copy(xtt[:, g * 512:(g + 1) * 512],
                               tr[:].bitcast(F32))
            else:
                nc.vector.tensor_copy(xtt[:, g * 512:(g + 1) * 512],
                                      tr[:].bitcast(F32))
        ps_t = tp.tile([RP, P], F32, tag="t")
        for k in range(KC):
            nc.tensor.matmul(ps_t[:], a_sb[:, k * RP:(k + 1) * RP],
                             xtt[:, k * P:(k + 1) * P],
                             start=(k == 0), stop=(k == KC - 1))
        nc.vector.tensor_copy(t_all[:, tslot * P:(tslot + 1) * P], ps_t[:])

    def emit_out(xt, t_all, tslot, bm, pso, oph, o_dram, t):
        """out tile = xt + t@Bmask, staged in half-tiles, DMA'd out."""
        for half in range(2):
            oh = oph.tile([P, 1024], F32, tag="oh")
            for n2 in range(2):
                n = half * 2 + n2
                ps_o = pso.tile([P, 512], F32, tag="o")
                nc.tensor.matmul(ps_o[:],
                                 t_all[:, tslot * P:(tslot + 1) * P],
                                 bm[:, n * 512:(n + 1) * 512],
                                 start=True, stop=True)
                eng = nc.vector
                eng.tensor_tensor(oh[:, n2 * 512:(n2 + 1) * 512],
                                  ps_o[:],
                                  xt[:, n * 512:(n + 1) * 512].bitcast(F32),
                                  op=mybir.AluOpType.add)
            nc.scalar.dma_start(
                out=o_dram[t * P:(t + 1) * P,
                           half * 1024:(half + 1) * 1024],
                in_=oh[:])

    with tile.TileContext(nc) as tc:
        with tc.tile_pool(name="const", bufs=1) as const:
            ones128 = const.tile([P, 1], F32R)
            nc.sync.dma_start(out=ones128[:], in_=onesd[:])
            id128 = const.tile([P, P], F32R)
            nc.sync.dma_start(out=id128[:], in_=idm[:])
            quarter = const.tile([B, 1], F32)
            nc.vector.memset(quarter[:], 1.0 / B)
            one1 = const.tile([1, 1], F32)
            nc.vector.memset(one1[:], 1.0)
            fsel_sb = const.tile([N_CORES, B], F32)
            nc.gpsimd.dma_start(out=fsel_sb[:], in_=fsel[:])
            maskc_sb = const.tile([1, 3 * RP], F32)
            nc.gpsimd.dma_start(out=maskc_sb[:], in_=maskc[:])
            b2_sb = const.tile([B, 1], F32)
            for p in range(B):
                nc.gpsimd.dma_start(out=b2_sb[p:p + 1, :], in_=b2[:])
            # per-core MLP slice params
            w1s_sb = const.tile([P, KC * HS], F32)
            nc.sync.dma_start(out=w1s_sb[:], in_=w1v[:])
            b1s_sb = const.tile([HS, 1], F32)
            nc.gpsimd.dma_start(out=b1s_sb[:], in_=b1s[:])
            w2s_sb = const.tile([HS, 1], F32)
            nc.gpsimd.dma_start(out=w2s_sb[:], in_=w2s[:])
            # LoRA A and B matrices, concatenated, bf16 (host-prepared)
            akb = const.tile([P, KC * RP], BF16)
            nc.sync.dma_start(out=akb[:], in_=akbd[:])
            avb = const.tile([P, KC * RP], BF16)
            nc.sync.dma_start(out=avb[:], in_=avbd[:])
            bkb = const.tile([RP, H], BF16)
            bvb = const.tile([RP, H], BF16)
            nc.gpsimd.dma_start(out=bkb[:], in_=bkbd[:])
            nc.gpsimd.dma_start(out=bvb[:], in_=bvbd[:])
            bmk = const.tile([RP, H], BF16)
            bmv = const.tile([RP, H], BF16)
            # (x@A)^T per tile, bf16
            tk_all = const.tile([RP, NT * P], BF16)
            tv_all = const.tile([RP, NT * P], BF16)
            kres = [const.tile([P, H], F32R, tag=f"kr{t}", name=f"kr{t}")
                    for t in range(NRES)]
            partial_sb = const.tile([1, H], F32)
            gath_sb = const.tile([N_CORES, H], F32)
            xmt_sb = const.tile([P, KC * B], F32)
            hb_sb = const.tile([HS, B], F32)
            psum_imp_sb = const.tile([B, 1], F32)
            imp_sb = const.tile([B, 1], F32)
            avg_sb = const.tile([1, 1], F32)
            s1_sb = const.tile([1, 1], F32)
            s2_sb = const.tile([1, 1], F32)
            m1_sb = const.tile([1, RP], F32)
            m2_sb = const.tile([1, RP], F32)
            mask_sb = const.tile([1, RP], F32)
            maskt_sb = const.tile([RP, 1], F32)

            vsp_cm = tc.tile_pool(name="vsp", bufs=4)
            vsp = vsp_cm.__enter__()
            oph_cm = tc.tile_pool(name="oph", bufs=3)
            oph = oph_cm.__enter__()
            vpre = []  # values tiles pre-loaded before the mask is ready

            # ---- Phase A1: stream keys, colsum all tiles, and
            # transpose+x@A for the non-resident (streamed) tiles ----
            with tc.tile_pool(name="ksp", bufs=2) as ksp, \
                 tc.tile_pool(name="xttp", bufs=2) as xttp, \
                 tc.tile_pool(name="csp", bufs=4, space="PSUM") as csp, \
                 tc.tile_pool(name="trp", bufs=2, space="PSUM") as trp, \
                 tc.tile_pool(name="tp", bufs=2, space="PSUM") as tp:
                cs = [csp.tile([1, 512], F32, tag="cs", name=f"cs{n}")
                      for n in range(4)]
                # streamed tiles interleaved with resident colsums so
                # the last colsum lands right after the last keys DMA
                nstream = NT - NRES
                order = []
                kq = list(range(NRES))
                for si, t in enumerate(range(NRES, NT)):
                    order.append(t)
                    take = (NRES * (si + 1)) // nstream - (NRES * si) // nstream
                    for _ in range(take):
                        order.append(kq.pop(0))
                order += kq
                first = {"v": True}
                ncs = {"n": 0}
                for t in order:
                    if t < NRES:
                        xt = kres[t]
                    else:
                        xt = ksp.tile([P, H], F32R, tag="ks")
                    nc.sync.dma_start(out=xt[:],
                                      in_=k_slab[t * P:(t + 1) * P, :])
                    ncs["n"] += 1
                    for n in range(4):
                        nc.tensor.matmul(
                            cs[n][:], ones128[:],
                            xt[:, n * 512:(n + 1) * 512],
                            start=first["v"], stop=(ncs["n"] == NT),
                            skip_group_check=True)
                    first["v"] = False
                    if t >= NRES:
                        xtt = xttp.tile([P, H], BF16, tag="xtt")
                        emit_txa(xt, xtt, trp, tp, akb, tk_all, t, id128)
                for n in range(4):
                    nc.scalar.copy(partial_sb[:, n * 512:(n + 1) * 512],
                                   cs[n][:])

                # ---- Collective 1: AllGather partial colsums ----
                with tc.tile_pool(name="dram", bufs=1, space="DRAM") as dram:
                    cc_in = dram.tile([1, H], F32)
                    cc_out = dram.tile([N_CORES, H], F32)
                    nc.gpsimd.dma_start(out=cc_in[:], in_=partial_sb[:])
                    nc.gpsimd.collective_compute(
                        "AllGather", mybir.AluOpType.bypass,
                        replica_groups=[list(range(N_CORES))],
                        ins=[cc_in.opt()], outs=[cc_out.opt()])
                    nc.gpsimd.dma_start(out=gath_sb[:], in_=cc_out[:])

                # ---- Phase A2 (overlaps collective): transpose+x@A for
                # the resident keys tiles ----
                for t in range(NRES):
                    xtt = xttp.tile([P, H], BF16, tag="xtt")
                    emit_txa(kres[t], xtt, trp, tp, akb, tk_all, t, id128)

                # ---- values pre-work (mask-independent) ----
                for t in range(NVPRE):
                    xt = vsp.tile([P, H], F32R, tag="vs")
                    nc.sync.dma_start(out=xt[:],
                                      in_=v_slab[t * P:(t + 1) * P, :])
                    vpre.append(xt)
                    xtt = xttp.tile([P, H], BF16, tag="xtt")
                    emit_txa(xt, xtt, trp, tp, avb, tv_all, t, id128)

                # ---- Phase B: MLP slice + collective 2 + mask ----
                with tc.tile_pool(name="mlp", bufs=2, space="PSUM") as mlp:
                    for k in range(KC):
                        ps = mlp.tile([P, B], F32, tag="xm")
                        nc.tensor.matmul(ps[:],
                                         gath_sb[:, k * P:(k + 1) * P],
                                         fsel_sb[:],
                                         start=True, stop=True)
                        nc.vector.tensor_copy(xmt_sb[:, k * B:(k + 1) * B],
                                              ps[:])
                    ps_h = mlp.tile([HS, B], F32, tag="h")
                    for k in range(KC):
                        nc.tensor.matmul(
                            ps_h[:], w1s_sb[:, k * HS:(k + 1) * HS],
                            xmt_sb[:, k * B:(k + 1) * B],
                            start=(k == 0), stop=(k == KC - 1))
                    nc.scalar.activation(hb_sb[:], ps_h[:],
                                         mybir.ActivationFunctionType.Relu,
                                         bias=b1s_sb[:, 0:1])
                    ps_p = mlp.tile([B, 1], F32, tag="p")
                    nc.tensor.matmul(ps_p[:], hb_sb[:], w2s_sb[:],
                                     start=True, stop=True)
                    nc.scalar.copy(psum_imp_sb[:], ps_p[:])

                    with tc.tile_pool(name="dram2", bufs=1,
                                      space="DRAM") as dram2:
                        cc2_in = dram2.tile([B, 1], F32)
                        cc2_out = dram2.tile([B, 1], F32)
                        nc.gpsimd.dma_start(out=cc2_in[:],
                                            in_=psum_imp_sb[:])
                        nc.gpsimd.collective_compute(
                            "AllReduce", mybir.AluOpType.add,
                            replica_groups=[list(range(N_CORES))],
                            ins=[cc2_in.opt()], outs=[cc2_out.opt()])
                        nc.gpsimd.dma_start(out=imp_sb[:], in_=cc2_out[:])

                    nc.vector.tensor_scalar(imp_sb[:], imp_sb[:],
                                            b2_sb[:, 0:1], None,
                                            op0=mybir.AluOpType.add)
                    nc.scalar.activation(imp_sb[:], imp_sb[:],
                                         mybir.ActivationFunctionType.Sigmoid)
                    ps_a = mlp.tile([1, 1], F32, tag="avg")
                    nc.tensor.matmul(ps_a[:], imp_sb[:], quarter[:],
                                     start=True, stop=True)
                    nc.scalar.copy(avg_sb[:], ps_a[:])
                    nc.vector.tensor_scalar(s1_sb[:], avg_sb[:], THR[0],
                                            None, op0=mybir.AluOpType.is_ge)
                    nc.vector.tensor_scalar(s2_sb[:], avg_sb[:], THR[1],
                                            None, op0=mybir.AluOpType.is_ge)
                    nc.vector.tensor_scalar(m1_sb[:], maskc_sb[:, RP:2 * RP],
                                            s1_sb[0:1, 0:1], None,
                                            op0=mybir.AluOpType.mult)
                    nc.vector.tensor_scalar(m2_sb[:],
                                            maskc_sb[:, 2 * RP:3 * RP],
                                            s2_sb[0:1, 0:1], None,
                                            op0=mybir.AluOpType.mult)
                    nc.vector.tensor_tensor(mask_sb[:], maskc_sb[:, 0:RP],
                                            m1_sb[:], op=mybir.AluOpType.add)
                    nc.vector.tensor_tensor(mask_sb[:], mask_sb[:], m2_sb[:],
                                            op=mybir.AluOpType.add)
                    ps_mt = mlp.tile([RP, 1], F32, tag="mt")
                    nc.tensor.matmul(ps_mt[:], mask_sb[:], one1[:],
                                     start=True, stop=True)
                    nc.scalar.copy(maskt_sb[:], ps_mt[:])
                    nc.vector.tensor_scalar(bmk[:], bkb[:],
                                            maskt_sb[:, 0:1], None,
                                            op0=mybir.AluOpType.mult)
                    nc.vector.tensor_scalar(bmv[:], bvb[:],
                                            maskt_sb[:, 0:1], None,
                                            op0=mybir.AluOpType.mult)

            # ---- Phase C: keys outputs ----
            with tc.tile_pool(name="krr", bufs=2) as krr, \
                 tc.tile_pool(name="kpso", bufs=2, space="PSUM") as kpso:
                for t in range(NT):
                    if t < NRES:
                        xt = kres[t]
                    else:
                        xt = krr.tile([P, H], F32R, tag="krr")
                        nc.gpsimd.dma_start(out=xt[:],
                                            in_=k_slab[t * P:(t + 1) * P, :])
                    emit_out(xt, tk_all, t, bmk, kpso, oph, ck_slab, t)

            # ---- Phase D: values outputs (and remaining pre-work) ----
            with tc.tile_pool(name="vxttp", bufs=2) as vxttp, \
                 tc.tile_pool(name="vtrp", bufs=2, space="PSUM") as vtrp, \
                 tc.tile_pool(name="vtp", bufs=2, space="PSUM") as vtp, \
                 tc.tile_pool(name="vpso", bufs=2, space="PSUM") as vpso:
                for t in range(NVPRE):
                    emit_out(vpre[t], tv_all, t, bmv, vpso, oph, cv_slab, t)
                for t in range(NVPRE, NT):
                    xt = vsp.tile([P, H], F32R, tag="vs")
                    nc.sync.dma_start(out=xt[:],
                                      in_=v_slab[t * P:(t + 1) * P, :])
                    xtt = vxttp.tile([P, H], BF16, tag="vxtt")
                    emit_txa(xt, xtt, vtrp, vtp, avb, tv_all, t, id128)
                    emit_out(xt, tv_all, t, bmv, vpso, oph, cv_slab, t)
            oph_cm.__exit__(None, None, None)
            vsp_cm.__exit__(None, None, None)

    nc.compile()
    return nc


def _get_program():
    if "nc" not in _cache:
        _cache["nc"] = _build_program()
    return _cache["nc"]


def _prep_in_maps(inputs):
    f32 = np.float32
    keys = np.asarray(inputs["keys"], dtype=f32)
    values = np.asarray(inputs["values"], dtype=f32)
    kf = np.ascontiguousarray(keys.reshape(B * S, H))
    vf = np.ascontiguousarray(values.reshape(B * S, H))
    scale = SCALING * RESIDUAL_SCALE

    def cat_a(a0, a1, a2):
        out = np.zeros((H, RP), dtype=f32)
        out[:, 0:4] = a0
        out[:, 4:12] = a1
        out[:, 12:28] = a2
        # chunk layout: [128, KC*RP], row p col k*RP+r = A[k*128+p, r]
        return np.ascontiguousarray(
            out.reshape(KC, P, RP).transpose(1, 0, 2).reshape(P, KC * RP)
        ).astype(BF16NP)

    def cat_b(b0, b1_, b2_):
        out = np.zeros((RP, H), dtype=f32)
        out[0:4, :] = b0
        out[4:12, :] = b1_
        out[12:28, :] = b2_
        return np.ascontiguousarray(out * scale).astype(BF16NP)

    acat_k = cat_a(inputs["kA0"], inputs["kA1"], inputs["kA2"])
    acat_v = cat_a(inputs["vA0"], inputs["vA1"], inputs["vA2"])
    bcat_k = cat_b(inputs["kB0"], inputs["kB1"], inputs["kB2"])
    bcat_v = cat_b(inputs["vB0"], inputs["vB1"], inputs["vB2"])

    fsel = np.zeros((N_CORES, B), dtype=f32)
    for c in range(N_CORES):
        fsel[c, c // 2] = 1.0 / S

    u = np.zeros((3, RP), dtype=f32)
    u[0, 0:4] = 1.0
    u[1, 4:12] = 1.0
    u[2, 12:28] = 1.0
    maskc = np.concatenate([u[0], u[1] - u[0], u[2] - u[1]]).astype(f32)

    w1 = np.ascontiguousarray(inputs["w1"], dtype=f32)
    b1 = np.asarray(inputs["b1"], dtype=f32).reshape(HH)
    w2 = np.asarray(inputs["w2"], dtype=f32).reshape(HH)

    common = {
        "b2": np.ascontiguousarray(
            np.asarray(inputs["b2"], dtype=f32).reshape(1, 1)),
        "akbd": acat_k, "avbd": acat_v,
        "bkbd": bcat_k, "bvbd": bcat_v,
        "fsel": fsel, "maskc": maskc.reshape(1, 3 * RP),
        "idm": np.eye(P, dtype=f32), "onesd": np.ones((P, 1), dtype=f32),
    }
    out = []
    for c in range(N_CORES):
        w1c = w1[:, c * HS:(c + 1) * HS]  # [H, HS]
        w1vc = np.ascontiguousarray(
            w1c.reshape(KC, P, HS).transpose(1, 0, 2).reshape(P, KC * HS))
        out.append(dict(
            common,
            k_slab=np.ascontiguousarray(kf[c * R:(c + 1) * R]),
            v_slab=np.ascontiguousarray(vf[c * R:(c + 1) * R]),
            w1v=w1vc,
            b1s=np.ascontiguousarray(b1[c * HS:(c + 1) * HS].reshape(HS, 1)),
            w2s=np.ascontiguousarray(w2[c * HS:(c + 1) * HS].reshape(HS, 1)),
        ))
    return out


def kernel(**inputs):
    in_maps = _prep_in_maps(inputs)
    nc = _get_program()
    res = run_bass_kernel_spmd(nc, in_maps, list(range(N_CORES)),
                               **_cache.get("run_kwargs", {}))
    _cache["last_result"] = res
    ck = np.concatenate([res.results[c]["ck_slab"] for c in range(N_CORES)],
                        axis=0).reshape(B, S, H)
    cv = np.concatenate([res.results[c]["cv_slab"] for c in range(N_CORES)],
                        axis=0).reshape(B, S, H)
    return ck, cv


# revision 35
# speedup vs baseline: 1.0161x; 1.0105x over previous
import sys

if "/opt/trn_rl_repo" not in sys.path:
    sys.path.insert(0, "/opt/trn_rl_repo")

import ml_dtypes
import numpy as np

import concourse.bacc as bacc
import concourse.bass as bass
import concourse.mybir as mybir
import concourse.tile as tile
from concourse.bass_utils import run_bass_kernel_spmd

# Problem constants (hardcoded per contract)
B, S, H = 4, 4096, 2048
HH = H // 2  # 1024
HS = HH // 8  # 128 hidden columns handled per core
RANKS = [4, 8, 16]
SCALING = 16.0 / max(RANKS)  # 1.0
RESIDUAL_SCALE = 1.0
THR = [0.3, 0.7]
N_CORES = 8
R = (B * S) // N_CORES  # 2048 rows per core
P = 128
NT = R // P  # 16 row tiles per core
KC = H // P  # 16 col chunks
RP = 32  # padded concat rank (4+8+16=28 -> 32)
NRES = 8  # keys row-tiles kept SBUF-resident for the phase-C residual add
NVPRE = 3  # values tiles whose transpose+x@A is emitted before the mask
F32 = mybir.dt.float32
F32R = mybir.dt.float32r
BF16 = mybir.dt.bfloat16
BF16NP = np.dtype(ml_dtypes.bfloat16)

_cache = {}


def _build_program():
    nc = bacc.Bacc("TRN2", target_bir_lowering=False, debug=False,
                   num_devices=N_CORES)

    k_slab = nc.dram_tensor("k_slab", [R, H], F32R, kind="ExternalInput").ap()
    v_slab = nc.dram_tensor("v_slab", [R, H], F32R, kind="ExternalInput").ap()
    w1v = nc.dram_tensor("w1v", [P, KC * HS], F32, kind="ExternalInput").ap()
    b1s = nc.dram_tensor("b1s", [HS, 1], F32, kind="ExternalInput").ap()
    w2s = nc.dram_tensor("w2s", [HS, 1], F32, kind="ExternalInput").ap()
    b2 = nc.dram_tensor("b2", [1, 1], F32, kind="ExternalInput").ap()
    akbd = nc.dram_tensor("akbd", [P, KC * RP], BF16,
                          kind="ExternalInput").ap()
    avbd = nc.dram_tensor("avbd", [P, KC * RP], BF16,
                          kind="ExternalInput").ap()
    bkbd = nc.dram_tensor("bkbd", [RP, H], BF16, kind="ExternalInput").ap()
    bvbd = nc.dram_tensor("bvbd", [RP, H], BF16, kind="ExternalInput").ap()
    fsel = nc.dram_tensor("fsel", [N_CORES, B], F32, kind="ExternalInput").ap()
    maskc = nc.dram_tensor("maskc", [1, 3 * RP], F32,
                           kind="ExternalInput").ap()
    idm = nc.dram_tensor("idm", [P, P], F32R, kind="ExternalInput").ap()
    onesd = nc.dram_tensor("onesd", [P, 1], F32R, kind="ExternalInput").ap()
    ck_slab = nc.dram_tensor("ck_slab", [R, H], F32, kind="ExternalOutput").ap()
    cv_slab = nc.dram_tensor("cv_slab", [R, H], F32, kind="ExternalOutput").ap()

    def emit_txa(xt, xtt, trp, tp, a_sb, t_all, tslot, id128):
        """Transpose xt and accumulate (x@A)^T into t_all[:, slot]."""
        for g in range(4):
            tr = trp.tile([P, 512], F32R, tag="tr")
            for j in range(4):
                k = g * 4 + j
                nc.tensor.transpose(tr[:, j * P:(j + 1) * P],
                                    xt[:, k * P:(k + 1) * P],
                                    id128[:])
            if g < 3:
                nc.scalar.copy(xtt[:, g * 512:(g + 1) * 512],
                               tr[:].bitcast(F32))
            else:
                nc.vector.tensor_copy(xtt[:, g * 512:(g + 1) * 512],
                                      tr[:].bitcast(F32))
        ps_t = tp.tile([RP, P], F32, tag="t")
        for k in range(KC):
            nc.tensor.matmul(ps_t[:], a_sb[:, k * RP:(k + 1) * RP],
                             xtt[:, k * P:(k + 1) * P],
                             start=(k == 0), stop=(k == KC - 1))
        nc.vector.tensor_copy(t_all[:, tslot * P:(tslot + 1) * P], ps_t[:])

    def emit_out(xt, t_all, tslot, bm, pso, oph, o_dram, t):
        """out tile = xt + t@Bmask, staged in half-tiles, DMA'd out."""
        for half in range(2):
            oh = oph.tile([P, 1024], F32, tag="oh")
            for n2 in range(2):
                n = half * 2 + n2
                ps_o = pso.tile([P, 512], F32, tag="o")
                nc.tensor.matmul(ps_o[:],
                                 t_all[:, tslot * P:(tslot + 1) * P],
                                 bm[:, n * 512:(n + 1) * 512],
                                 start=True, stop=True)
                eng = nc.vector
                eng.tensor_tensor(oh[:, n2 * 512:(n2 + 1) * 512],
                                  ps_o[:],
                                  xt[:, n * 512:(n + 1) * 512].bitcast(F32),
                                  op=mybir.AluOpType.add)
            nc.scalar.dma_start(
                out=o_dram[t * P:(t + 1) * P,
                           half * 1024:(half + 1) * 1024],
                in_=oh[:])

    with tile.TileContext(nc) as tc:
        with tc.tile_pool(name="const", bufs=1) as const:
            ones128 = const.tile([P, 1], F32R)
            nc.sync.dma_start(out=ones128[:], in_=onesd[:])
            id128 = const.tile([P, P], F32R)
            nc.sync.dma_start(out=id128[:], in_=idm[:])
            quarter = const.tile([B, 1], F32)
            nc.vector.memset(quarter[:], 1.0 / B)
            one1 = const.tile([1, 1], F32)
            nc.vector.memset(one1[:], 1.0)
            fsel_sb = const.tile([N_CORES, B], F32)
            nc.gpsimd.dma_start(out=fsel_sb[:], in_=fsel[:])
            maskc_sb = const.tile([1, 3 * RP], F32)
            nc.gpsimd.dma_start(out=maskc_sb[:], in_=maskc[:])
            b2_sb = const.tile([B, 1], F32)
            for p in range(B):
                nc.gpsimd.dma_start(out=b2_sb[p:p + 1, :], in_=b2[:])
            # per-core MLP slice params
            w1s_sb = const.tile([P, KC * HS], F32)
            nc.sync.dma_start(out=w1s_sb[:], in_=w1v[:])
            b1s_sb = const.tile([HS, 1], F32)
            nc.gpsimd.dma_start(out=b1s_sb[:], in_=b1s[:])
            w2s_sb = const.tile([HS, 1], F32)
            nc.gpsimd.dma_start(out=w2s_sb[:], in_=w2s[:])
            # LoRA A and B matrices, concatenated, bf16 (host-prepared)
            akb = const.tile([P, KC * RP], BF16)
            nc.sync.dma_start(out=akb[:], in_=akbd[:])
            avb = const.tile([P, KC * RP], BF16)
            nc.sync.dma_start(out=avb[:], in_=avbd[:])
            bkb = const.tile([RP, H], BF16)
            bvb = const.tile([RP, H], BF16)
            nc.gpsimd.dma_start(out=bkb[:], in_=bkbd[:])
            nc.gpsimd.dma_start(out=bvb[:], in_=bvbd[:])
            bmk = const.tile([RP, H], BF16)
            bmv = const.tile([RP, H], BF16)
            # (x@A)^T per tile, bf16
            tk_all = const.tile([RP, NT * P], BF16)
            tv_all = const.tile([RP, NT * P], BF16)
            kres = [const.tile([P, H], F32R, tag=f"kr{t}", name=f"kr{t}")
                    for t in range(NRES)]
            partial_sb = const.tile([1, H], F32)
            partial2_sb = const.tile([1, H], F32)
            gath_sb = const.tile([N_CORES, H], F32)
            gath2_sb = const.tile([N_CORES, H], F32)
            xmt_sb = const.tile([P, KC * B], F32)
            hb_sb = const.tile([HS, B], F32)
            psum_imp_sb = const.tile([B, 1], F32)
            imp_sb = const.tile([B, 1], F32)
            avg_sb = const.tile([1, 1], F32)
            s1_sb = const.tile([1, 1], F32)
            s2_sb = const.tile([1, 1], F32)
            m1_sb = const.tile([1, RP], F32)
            m2_sb = const.tile([1, RP], F32)
            mask_sb = const.tile([1, RP], F32)
            maskt_sb = const.tile([RP, 1], F32)

            vsp_cm = tc.tile_pool(name="vsp", bufs=4)
            vsp = vsp_cm.__enter__()
            oph_cm = tc.tile_pool(name="oph", bufs=4)
            oph = oph_cm.__enter__()
            vpre = []  # values tiles pre-loaded before the mask is ready

            # ---- Phase A1: stream keys, colsum all tiles, and
            # transpose+x@A for the non-resident (streamed) tiles ----
            with tc.tile_pool(name="ksp", bufs=2) as ksp, \
                 tc.tile_pool(name="xttp", bufs=2) as xttp, \
                 tc.tile_pool(name="csp", bufs=4, space="PSUM") as csp, \
                 tc.tile_pool(name="trp", bufs=2, space="PSUM") as trp, \
                 tc.tile_pool(name="tp", bufs=2, space="PSUM") as tp:
                cs = [csp.tile([1, 512], F32, tag="cs", name=f"cs{n}")
                      for n in range(4)]
                # streamed tiles interleaved with resident colsums so
                # the last colsum lands right after the last keys DMA
                nstream = NT - NRES
                order = []
                kq = list(range(NRES))
                for si, t in enumerate(range(NRES, NT)):
                    order.append(t)
                    take = (NRES * (si + 1)) // nstream - (NRES * si) // nstream
                    for _ in range(take):
                        order.append(kq.pop(0))
                order += kq
                dram_cm = tc.tile_pool(name="dram", bufs=1, space="DRAM")
                dram = dram_cm.__enter__()
                cc_in = [dram.tile([1, H], F32, name=f"ccin{h}")
                         for h in range(2)]
                cc_out = [dram.tile([N_CORES, H], F32, name=f"ccout{h}")
                          for h in range(2)]

                def launch_cc(h, par):
                    for n in range(4):
                        nc.scalar.copy(par[:, n * 512:(n + 1) * 512],
                                       cs[n][:])
                    nc.gpsimd.dma_start(out=cc_in[h][:], in_=par[:])
                    nc.gpsimd.collective_compute(
                        "AllGather", mybir.AluOpType.bypass,
                        replica_groups=[list(range(N_CORES))],
                        ins=[cc_in[h].opt()], outs=[cc_out[h].opt()])

                ncs = {"n": 0}
                for t in order:
                    if t < NRES:
                        xt = kres[t]
                    else:
                        xt = ksp.tile([P, H], F32R, tag="ks")
                    nc.sync.dma_start(out=xt[:],
                                      in_=k_slab[t * P:(t + 1) * P, :])
                    ncs["n"] += 1
                    for n in range(4):
                        nc.tensor.matmul(
                            cs[n][:], ones128[:],
                            xt[:, n * 512:(n + 1) * 512],
                            start=(ncs["n"] in (1, NT // 2 + 1)),
                            stop=(ncs["n"] in (NT // 2, NT)),
                            skip_group_check=True)
                    if ncs["n"] == NT // 2:
                        launch_cc(0, partial_sb)
                    elif ncs["n"] == NT:
                        launch_cc(1, partial2_sb)
                    if t >= NRES:
                        xtt = xttp.tile([P, H], BF16, tag="xtt")
                        emit_txa(xt, xtt, trp, tp, akb, tk_all, t, id128)

                nc.gpsimd.dma_start(out=gath_sb[:], in_=cc_out[0][:])
                nc.gpsimd.dma_start(out=gath2_sb[:], in_=cc_out[1][:])
                nc.vector.tensor_tensor(gath_sb[:], gath_sb[:],
                                        gath2_sb[:],
                                        op=mybir.AluOpType.add)
                dram_cm.__exit__(None, None, None)

                # ---- Phase A2 (overlaps collective): transpose+x@A for
                # the resident keys tiles ----
                for t in range(NRES):
                    xtt = xttp.tile([P, H], BF16, tag="xtt")
                    emit_txa(kres[t], xtt, trp, tp, akb, tk_all, t, id128)

                # ---- values pre-work (mask-independent) ----
                for t in range(NVPRE):
                    xt = vsp.tile([P, H], F32R, tag="vs")
                    nc.sync.dma_start(out=xt[:],
                                      in_=v_slab[t * P:(t + 1) * P, :])
                    vpre.append(xt)
                    xtt = xttp.tile([P, H], BF16, tag="xtt")
                    emit_txa(xt, xtt, trp, tp, avb, tv_all, t, id128)

                # ---- Phase B: MLP slice + collective 2 + mask ----
                with tc.tile_pool(name="mlp", bufs=2, space="PSUM") as mlp:
                    for k in range(KC):
                        ps = mlp.tile([P, B], F32, tag="xm")
                        nc.tensor.matmul(ps[:],
                                         gath_sb[:, k * P:(k + 1) * P],
                                         fsel_sb[:],
                                         start=True, stop=True)
                        nc.vector.tensor_copy(xmt_sb[:, k * B:(k + 1) * B],
                                              ps[:])
                    ps_h = mlp.tile([HS, B], F32, tag="h")
                    for k in range(KC):
                        nc.tensor.matmul(
                            ps_h[:], w1s_sb[:, k * HS:(k + 1) * HS],
                            xmt_sb[:, k * B:(k + 1) * B],
                            start=(k == 0), stop=(k == KC - 1))
                    nc.scalar.activation(hb_sb[:], ps_h[:],
                                         mybir.ActivationFunctionType.Relu,
                                         bias=b1s_sb[:, 0:1])
                    ps_p = mlp.tile([B, 1], F32, tag="p")
                    nc.tensor.matmul(ps_p[:], hb_sb[:], w2s_sb[:],
                                     start=True, stop=True)
                    nc.scalar.copy(psum_imp_sb[:], ps_p[:])

                    with tc.tile_pool(name="dram2", bufs=1,
                                      space="DRAM") as dram2:
                        cc2_in = dram2.tile([B, 1], F32)
                        cc2_out = dram2.tile([B, 1], F32)
                        nc.gpsimd.dma_start(out=cc2_in[:],
                                            in_=psum_imp_sb[:])
                        nc.gpsimd.collective_compute(
                            "AllReduce", mybir.AluOpType.add,
                            replica_groups=[list(range(N_CORES))],
                            ins=[cc2_in.opt()], outs=[cc2_out.opt()])
                        nc.gpsimd.dma_start(out=imp_sb[:], in_=cc2_out[:])

                    nc.vector.tensor_scalar(imp_sb[:], imp_sb[:],
                                            b2_sb[:, 0:1], None,
                                            op0=mybir.AluOpType.add)
                    nc.scalar.activation(imp_sb[:], imp_sb[:],
                                         mybir.ActivationFunctionType.Sigmoid)
                    ps_a = mlp.tile([1, 1], F32, tag="avg")
                    nc.tensor.matmul(ps_a[:], imp_sb[:], quarter[:],
                                     start=True, stop=True)
                    nc.scalar.copy(avg_sb[:], ps_a[:])
                    nc.vector.tensor_scalar(s1_sb[:], avg_sb[:], THR[0],
                                            None, op0=mybir.AluOpType.is_ge)
                    nc.vector.tensor_scalar(s2_sb[:], avg_sb[:], THR[1],
                                            None, op0=mybir.AluOpType.is_ge)
                    nc.vector.tensor_scalar(m1_sb[:], maskc_sb[:, RP:2 * RP],
                                            s1_sb[0:1, 0:1], None,
                                            op0=mybir.AluOpType.mult)
                    nc.vector.tensor_scalar(m2_sb[:],
                                            maskc_sb[:, 2 * RP:3 * RP],
                                            s2_sb[0:1, 0:1], None,
                                            op0=mybir.AluOpType.mult)
                    nc.vector.tensor_tensor(mask_sb[:], maskc_sb[:, 0:RP],
                                            m1_sb[:], op=mybir.AluOpType.add)
                    nc.vector.tensor_tensor(mask_sb[:], mask_sb[:], m2_sb[:],
                                            op=mybir.AluOpType.add)
                    ps_mt = mlp.tile([RP, 1], F32, tag="mt")
                    nc.tensor.matmul(ps_mt[:], mask_sb[:], one1[:],
                                     start=True, stop=True)
                    nc.scalar.copy(maskt_sb[:], ps_mt[:])
                    nc.vector.tensor_scalar(bmk[:], bkb[:],
                                            maskt_sb[:, 0:1], None,
                                            op0=mybir.AluOpType.mult)
                    nc.vector.tensor_scalar(bmv[:], bvb[:],
                                            maskt_sb[:, 0:1], None,
                                            op0=mybir.AluOpType.mult)

            # ---- Phase C: keys outputs ----
            with tc.tile_pool(name="krr", bufs=2) as krr, \
                 tc.tile_pool(name="kpso", bufs=2, space="PSUM") as kpso:
                for t in range(NT):
                    if t < NRES:
                        xt = kres[t]
                    else:
                        xt = krr.tile([P, H], F32R, tag="krr")
                        nc.gpsimd.dma_start(out=xt[:],
                                            in_=k_slab[t * P:(t + 1) * P, :])
                    emit_out(xt, tk_all, t, bmk, kpso, oph, ck_slab, t)

            # ---- Phase D: values outputs (and remaining pre-work) ----
            with tc.tile_pool(name="vxttp", bufs=2) as vxttp, \
                 tc.tile_pool(name="vtrp", bufs=2, space="PSUM") as vtrp, \
                 tc.tile_pool(name="vtp", bufs=2, space="PSUM") as vtp, \
                 tc.tile_pool(name="vpso", bufs=2, space="PSUM") as vpso:
                for t in range(NVPRE):
                    emit_out(vpre[t], tv_all, t, bmv, vpso, oph, cv_slab, t)
                for t in range(NVPRE, NT):
                    xt = vsp.tile([P, H], F32R, tag="vs")
                    nc.sync.dma_start(out=xt[:],
                                      in_=v_slab[t * P:(t + 1) * P, :])
                    xtt = vxttp.tile([P, H], BF16, tag="vxtt")
                    emit_txa(xt, xtt, vtrp, vtp, avb, tv_all, t, id128)
                    emit_out(xt, tv_all, t, bmv, vpso, oph, cv_slab, t)
            oph_cm.__exit__(None, None, None)
            vsp_cm.__exit__(None, None, None)

    nc.compile()
    return nc


def _get_program():
    if "nc" not in _cache:
        _cache["nc"] = _build_program()
    return _cache["nc"]


def _prep_in_maps(inputs):
    f32 = np.float32
    keys = np.asarray(inputs["keys"], dtype=f32)
    values = np.asarray(inputs["values"], dtype=f32)
    kf = np.ascontiguousarray(keys.reshape(B * S, H))
    vf = np.ascontiguousarray(values.reshape(B * S, H))
    scale = SCALING * RESIDUAL_SCALE

    def cat_a(a0, a1, a2):
        out = np.zeros((H, RP), dtype=f32)
        out[:, 0:4] = a0
        out[:, 4:12] = a1
        out[:, 12:28] = a2
        # chunk layout: [128, KC*RP], row p col k*RP+r = A[k*128+p, r]
        return np.ascontiguousarray(
            out.reshape(KC, P, RP).transpose(1, 0, 2).reshape(P, KC * RP)
        ).astype(BF16NP)

    def cat_b(b0, b1_, b2_):
        out = np.zeros((RP, H), dtype=f32)
        out[0:4, :] = b0
        out[4:12, :] = b1_
        out[12:28, :] = b2_
        return np.ascontiguousarray(out * scale).astype(BF16NP)

    acat_k = cat_a(inputs["kA0"], inputs["kA1"], inputs["kA2"])
    acat_v = cat_a(inputs["vA0"], inputs["vA1"], inputs["vA2"])
    bcat_k = cat_b(inputs["kB0"], inputs["kB1"], inputs["kB2"])
    bcat_v = cat_b(inputs["vB0"], inputs["vB1"], inputs["vB2"])

    fsel = np.zeros((N_CORES, B), dtype=f32)
    for c in range(N_CORES):
        fsel[c, c // 2] = 1.0 / S

    u = np.zeros((3, RP), dtype=f32)
    u[0, 0:4] = 1.0
    u[1, 4:12] = 1.0
    u[2, 12:28] = 1.0
    maskc = np.concatenate([u[0], u[1] - u[0], u[2] - u[1]]).astype(f32)

    w1 = np.ascontiguousarray(inputs["w1"], dtype=f32)
    b1 = np.asarray(inputs["b1"], dtype=f32).reshape(HH)
    w2 = np.asarray(inputs["w2"], dtype=f32).reshape(HH)

    common = {
        "b2": np.ascontiguousarray(
            np.asarray(inputs["b2"], dtype=f32).reshape(1, 1)),
        "akbd": acat_k, "avbd": acat_v,
        "bkbd": bcat_k, "bvbd": bcat_v,
        "fsel": fsel, "maskc": maskc.reshape(1, 3 * RP),
        "idm": np.eye(P, dtype=f32), "onesd": np.ones((P, 1), dtype=f32),
    }
    out = []
    for c in range(N_CORES):
        w1c = w1[:, c * HS:(c + 1) * HS]  # [H, HS]
        w1vc = np.ascontiguousarray(
            w1c.reshape(KC, P, HS).transpose(1, 0, 2).reshape(P, KC * HS))
        out.append(dict(
            common,
            k_slab=np.ascontiguousarray(kf[c * R:(c + 1) * R]),
            v_slab=np.ascontiguousarray(vf[c * R:(c + 1) * R]),
            w1v=w1vc,
            b1s=np.ascontiguousarray(b1[c * HS:(c + 1) * HS].reshape(HS, 1)),
            w2s=np.ascontiguousarray(w2[c * HS:(c + 1) * HS].reshape(HS, 1)),
        ))
    return out


def kernel(**inputs):
    in_maps = _prep_in_maps(inputs)
    nc = _get_program()
    res = run_bass_kernel_spmd(nc, in_maps, list(range(N_CORES)),
                               **_cache.get("run_kwargs", {}))
    _cache["last_result"] = res
    ck = np.concatenate([res.results[c]["ck_slab"] for c in range(N_CORES)],
                        axis=0).reshape(B, S, H)
    cv = np.concatenate([res.results[c]["cv_slab"] for c in range(N_CORES)],
                        axis=0).reshape(B, S, H)
    return ck, cv
